# revision 52
# baseline (speedup 1.0000x reference)
"""Trainium2 Bass kernel for nn_Analogy_RE_Model (NCE + pairwise-BCE loss).

Strategy (8 NeuronCores, shard positive-row axis i):
  - Entire cost is t3[i,j] = sum_d w3_d * |pos[i,d] - allv[j,d]|  (512x1024x512).
    Computed as: one DVE tensor_scalar op per (i, d-chunk):
        X = abs_max(bst_chunk - ps_i, 0) = |allv_T - pos_i|   (bf16, 4x mode)
    then TensorE reduces over d with w3 as stationary weights (M=1 matmuls,
    f32 PSUM accumulation, output row = i).
  - NCE part (cos matrices, exp/log) in f32/bf16 on the side: raw bf16 grams
    via matmul, normalization folded in as inv_i (per-partition scalar) and
    inv_j (DMA-broadcast row) scales.
  - Each core outputs [64, 2]: per-i loss1 partial and per-i BCE sum; host
    sums the 8 cores' partials (the "all-reduce" of a scalar loss).
"""

import sys

sys.path.insert(0, "/opt/trn_rl_repo")

import numpy as np

N, M, D = 512, 512, 512
NJ = N + M
NCORES = 8
IL = N // NCORES  # 64 local i rows per core
DT = D // 128  # 4 contraction chunks
EPS = 1e-5
COS_EPS = 1e-8

_CACHE: dict = {}


def _build_program(reps=1, skip_mm=False, skip_x=False, act_every=3, x_bufs=10):
    """K term via the min identity: w|a-b| = wa + wb + s*min(|w|a, |w|b)
    with s = -2 for w>=0 and +2 for w<0 (folded into the per-partition PE
    reduce weights). One full-width DVE min op per (i, d-chunk).

    reps>1 repeats the heavy phase in-NEFF (for slope-based timing only).
    """
    from concourse import bacc, mybir, tile

    f32 = mybir.dt.float32
    bf16 = mybir.dt.bfloat16
    Alu = mybir.AluOpType
    Act = mybir.ActivationFunctionType

    nc = bacc.Bacc("TRN2", target_bir_lowering=False, debug=False)

    pos_d = nc.dram_tensor("pos", [N, D], f32, kind="ExternalInput").ap()
    neg_d = nc.dram_tensor("neg", [M, D], f32, kind="ExternalInput").ap()
    posl_d = nc.dram_tensor("pos_loc", [IL, D], f32, kind="ExternalInput").ap()
    bst_d = nc.dram_tensor("bst", [D, NJ], bf16, kind="ExternalInput").ap()
    gst_d = nc.dram_tensor("gst", [D, NJ], bf16, kind="ExternalInput").ap()
    pst_d = nc.dram_tensor("pst", [D, IL], bf16, kind="ExternalInput").ap()
    pstf_d = nc.dram_tensor("pstf", [D, IL], f32, kind="ExternalInput").ap()
    pstfn_d = nc.dram_tensor("pstfn", [D, IL], f32, kind="ExternalInput").ap()
    w3t_d = nc.dram_tensor("w3t", [D, 1], bf16, kind="ExternalInput").ap()
    al_d = nc.dram_tensor("alpha_l", [IL, 1], f32, kind="ExternalInput").ap()
    beta_d = nc.dram_tensor("beta", [NJ, 1], f32, kind="ExternalInput").ap()
    out_nce_d = nc.dram_tensor("out_nce", [IL, 1], f32, kind="ExternalOutput").ap()
    out_bce_d = nc.dram_tensor("out_bce", [128, 8], f32, kind="ExternalOutput").ap()

    with tile.TileContext(nc) as tc:
        with (
            tc.tile_pool(name="const", bufs=1) as cp,
            tc.tile_pool(name="xp", bufs=x_bufs) as xp,
            tc.tile_pool(name="sm", bufs=1) as sm,
            tc.tile_pool(name="scr", bufs=2) as scr,
            tc.tile_pool(name="psum", bufs=1, space="PSUM") as pp,
            tc.tile_pool(name="dram", bufs=1, space="DRAM") as dp,
        ):
            # ---- constant loads ----
            bst_t = []
            pst_t = []
            pstf_t = []
            pstfn_t = []
            w3_t = []
            gst_t = []
            for dt in range(DT):
                bt = cp.tile([128, NJ], bf16, tag=f"bst{dt}")
                nc.sync.dma_start(out=bt, in_=bst_d[dt * 128 : (dt + 1) * 128, :])
                bst_t.append(bt)
                gt = cp.tile([128, NJ], bf16, tag=f"gst{dt}")
                nc.sync.dma_start(out=gt, in_=gst_d[dt * 128 : (dt + 1) * 128, :])
                gst_t.append(gt)
                pt = cp.tile([128, IL], bf16, tag=f"pst{dt}")
                nc.sync.dma_start(out=pt, in_=pst_d[dt * 128 : (dt + 1) * 128, :])
                pst_t.append(pt)
                pft = cp.tile([128, IL], f32, tag=f"pstf{dt}")
                nc.sync.dma_start(out=pft, in_=pstf_d[dt * 128 : (dt + 1) * 128, :])
                pstf_t.append(pft)
                pfn = cp.tile([128, IL], f32, tag=f"pstfn{dt}")
                nc.sync.dma_start(out=pfn, in_=pstfn_d[dt * 128 : (dt + 1) * 128, :])
                pstfn_t.append(pfn)
                wt = cp.tile([128, 1], bf16, tag=f"w3{dt}")
                nc.sync.dma_start(out=wt, in_=w3t_d[dt * 128 : (dt + 1) * 128, :])
                w3_t.append(wt)

            bst_v = bst_t
            pstf_v = pstf_t

            # alpha replicated across j-partitions; beta in [j_p, jt] layout
            alpha_rep = cp.tile([128, IL], f32, tag="alpha_rep")
            nc.sync.dma_start(
                out=alpha_rep,
                in_=al_d.squeeze(1).unsqueeze(0).broadcast_to((128, IL)),
            )
            beta_t = cp.tile([128, 8], f32, tag="beta_t")
            nc.sync.dma_start(
                out=beta_t, in_=beta_d.squeeze(1).rearrange("(t p) -> p t", p=128)
            )
            alpha_v = alpha_rep
            beta_v = beta_t

            # ---- K loop: psumKj[j_p, jt*IL + i] = sum_d w3_d |pos_i - allv_j| ----
            # X = |allv_T - pos_i| per (i, d-chunk); TensorE contracts the
            # 128-d chunk with w3 (X stationary, w3 moving, N=1).
            psumKj = pp.tile([128, 512], f32, tag="K")
            import contextlib

            # hardware loop for big timing reps; python-unroll small reps
            hw_loop = reps > 8
            loop_ctx = (
                tc.For_i(0, reps, 1) if hw_loop else contextlib.nullcontext()
            )
            with loop_ctx:
              for _rep in range(1 if hw_loop else reps):
                Xs0 = None
                for i in range(IL):
                    if skip_x and Xs0 is not None:
                        Xs = Xs0
                    else:
                        # X = relu(b'' - p''_i); every act_every-th i-block on
                        # ScalarE (Relu with per-partition bias), rest on DVE
                        use_act = act_every and (i % act_every == act_every - 1)
                        Xs = []
                        for dt in range(DT):
                            X = xp.tile([128, NJ], bf16, tag="X")
                            if use_act:
                                nc.scalar.activation(
                                    out=X,
                                    in_=bst_v[dt],
                                    func=Act.Relu,
                                    bias=pstfn_t[dt][:, i : i + 1],
                                )
                            else:
                                nc.vector.tensor_scalar(
                                    out=X,
                                    in0=bst_v[dt],
                                    scalar1=pstf_v[dt][:, i : i + 1],
                                    scalar2=0.0,
                                    op0=Alu.subtract,
                                    op1=Alu.max,
                                )
                            Xs.append(X)
                        Xs0 = Xs
                    if skip_mm and i > 0:
                        continue
                    for jt in range(8):
                        for dt in range(DT):
                            nc.tensor.matmul(
                                psumKj[:, jt * IL + i : jt * IL + i + 1],
                                lhsT=Xs[dt][:, jt * 128 : (jt + 1) * 128],
                                rhs=w3_t[dt],
                                start=(dt == 0),
                                stop=(dt == DT - 1),
                            )

            # ---- norms: inv_j for all pos rows and neg rows ----
            invp_dram = dp.tile([N, 1], f32, tag="invp_d")
            invn_dram = dp.tile([M, 1], f32, tag="invn_d")
            for src_d, inv_dram in ((pos_d, invp_dram), (neg_d, invn_dram)):
                for k in range(4):
                    rows = src_d[k * 128 : (k + 1) * 128, :]
                    rt = scr.tile([128, D], f32, tag="rowload")
                    nc.sync.dma_start(out=rt, in_=rows)
                    ss = sm.tile([128, 1], f32, tag=f"ss{id(inv_dram)}_{k}")
                    dump = scr.tile([128, D], bf16, tag="actdump")
                    nc.scalar.activation(
                        out=dump, in_=rt, func=Act.Square, accum_out=ss
                    )
                    nrm = sm.tile([128, 1], f32, tag=f"nrm{id(inv_dram)}_{k}")
                    nc.scalar.activation(out=nrm, in_=ss, func=Act.Sqrt)
                    nc.vector.tensor_scalar(
                        out=nrm,
                        in0=nrm,
                        scalar1=COS_EPS,
                        scalar2=None,
                        op0=Alu.max,
                    )
                    inv = sm.tile([128, 1], f32, tag=f"inv{id(inv_dram)}_{k}")
                    nc.vector.reciprocal(out=inv, in_=nrm)
                    nc.sync.dma_start(
                        out=inv_dram[k * 128 : (k + 1) * 128, :], in_=inv
                    )

            invp_rep = cp.tile([IL, N], f32, tag="invp_rep")
            nc.sync.dma_start(
                out=invp_rep,
                in_=invp_dram.squeeze(1).unsqueeze(0).broadcast_to((IL, N)),
            )
            invn_rep = cp.tile([IL, M], f32, tag="invn_rep")
            nc.sync.dma_start(
                out=invn_rep,
                in_=invn_dram.squeeze(1).unsqueeze(0).broadcast_to((IL, M)),
            )
            invp_v = invp_rep
            invn_v = invn_rep

            # inv for the local i rows
            plt = cp.tile([IL, D], f32, tag="posl")
            nc.sync.dma_start(out=plt, in_=posl_d)
            ssl = sm.tile([IL, 1], f32, tag="ssl")
            dumpl = scr.tile([IL, D], bf16, tag="actdump_l")
            nc.scalar.activation(out=dumpl, in_=plt, func=Act.Square, accum_out=ssl)
            nrml = sm.tile([IL, 1], f32, tag="nrml")
            nc.scalar.activation(out=nrml, in_=ssl, func=Act.Sqrt)
            nc.vector.tensor_scalar(
                out=nrml, in0=nrml, scalar1=COS_EPS, scalar2=None, op0=Alu.max
            )
            invl = sm.tile([IL, 1], f32, tag="invl")
            nc.vector.reciprocal(out=invl, in_=nrml)

            # ---- raw grams via bf16 matmul ----
            G_pp = pp.tile([IL, N], f32, tag="Gpp")
            G_pn = pp.tile([IL, M], f32, tag="Gpn")
            for dt in range(DT):
                nc.tensor.matmul(
                    G_pp,
                    lhsT=pst_t[dt],
                    rhs=gst_t[dt][:, 0:N],
                    start=(dt == 0),
                    stop=(dt == DT - 1),
                )
            for dt in range(DT):
                nc.tensor.matmul(
                    G_pn,
                    lhsT=pst_t[dt],
                    rhs=gst_t[dt][:, N:NJ],
                    start=(dt == 0),
                    stop=(dt == DT - 1),
                )

            # cos matrices: cos = G * inv_i * inv_j
            cos_pp = sm.tile([IL, N], f32, tag="cospp")
            cos_sum = sm.tile([IL, 1], f32, tag="cossum")
            nc.vector.scalar_tensor_tensor(
                out=cos_pp,
                in0=G_pp,
                scalar=invl,
                in1=invp_v,
                op0=Alu.mult,
                op1=Alu.mult,
                accum_out=cos_sum,
            )
            cos_pn = sm.tile([IL, M], f32, tag="cospn")
            nc.vector.scalar_tensor_tensor(
                out=cos_pn,
                in0=G_pn,
                scalar=invl,
                in1=invn_v,
                op0=Alu.mult,
                op1=Alu.mult,
            )

            # deno_i = sum_j exp(cos_pn)
            deno = sm.tile([IL, 1], f32, tag="deno")
            dump2 = scr.tile([IL, M], bf16, tag="actdump_e")
            nc.scalar.activation(
                out=dump2, in_=cos_pn, func=Act.Exp, accum_out=deno
            )
            # logit_p = exp(cos_pp)
            logit_p = sm.tile([IL, N], f32, tag="logitp")
            nc.scalar.activation(out=logit_p, in_=cos_pp, func=Act.Exp)
            # biasv = deno + EPS
            biasv = sm.tile([IL, 1], f32, tag="biasv")
            nc.vector.tensor_scalar(
                out=biasv, in0=deno, scalar1=EPS, scalar2=None, op0=Alu.add
            )
            # lgsum_i = sum_j log(logit_p + deno_i + EPS)
            lgsum = sm.tile([IL, 1], f32, tag="lgsum")
            dump3 = scr.tile([IL, N], bf16, tag="actdump_ln")
            nc.scalar.activation(
                out=dump3,
                in_=logit_p,
                func=Act.Ln,
                bias=biasv,
                accum_out=lgsum,
            )

            # ---- BCE tail (j-partition layout) ----
            # logits = K + beta_j (per-partition) + alpha_i (replicated row);
            # bce_cols[j_p, jt] = sum_i softplus(+-logits)
            bce_cols = sm.tile([128, 8], f32, tag="bce_cols")
            for jt in range(8):
                Ljt = sm.tile([128, IL], f32, tag=f"L{jt % 2}")
                nc.vector.scalar_tensor_tensor(
                    out=Ljt,
                    in0=psumKj[:, jt * IL : (jt + 1) * IL],
                    scalar=beta_v[:, jt : jt + 1],
                    in1=alpha_v,
                    op0=Alu.add,
                    op1=Alu.add,
                )
                # softplus(s*L) = ln(exp(s*L) + 1)
                eL = sm.tile([128, IL], f32, tag=f"eL{jt % 2}")
                nc.scalar.activation(
                    out=eL,
                    in_=Ljt,
                    func=Act.Exp,
                    scale=(-1.0 if jt < 4 else 1.0),
                )
                dumps = scr.tile([128, IL], bf16, tag="actdump_sp")
                nc.scalar.activation(
                    out=dumps,
                    in_=eL,
                    func=Act.Ln,
                    bias=1.0,
                    accum_out=bce_cols[:, jt : jt + 1],
                )
            nc.sync.dma_start(out=out_bce_d, in_=bce_cols)

            # ---- NCE output ----
            out_sb = sm.tile([IL, 1], f32, tag="outsb")
            nc.vector.tensor_tensor(
                out=out_sb, in0=lgsum, in1=cos_sum, op=Alu.subtract
            )
            nc.sync.dma_start(out=out_nce_d, in_=out_sb)

    nc.compile()
    return nc


def _prep_inputs(tensor_positive, tensor_negative, linear_w, linear_b):
    import ml_dtypes

    bf = ml_dtypes.bfloat16
    pos = np.asarray(tensor_positive, np.float32)
    neg = np.asarray(tensor_negative, np.float32)
    w = np.asarray(linear_w, np.float32)[0]
    b = np.float32(np.asarray(linear_b, np.float32)[0])
    w1, w2, w3 = w[:D], w[D : 2 * D], w[2 * D :]

    allv = np.concatenate([pos, neg], axis=0)  # [NJ, D]
    aw3 = np.abs(w3)
    bst = np.ascontiguousarray((allv * aw3).T).astype(bf)  # [D, NJ]
    gst = np.ascontiguousarray(allv.T).astype(bf)  # raw, for the grams
    # X = relu(b''-p'') with min(b,p) = b - relu(b-p): the rank-1 b-term
    # folds into beta (allv@(w2-w3)); PE weights flip sign vs the min form
    w3t = np.where(w3 >= 0, 2.0, -2.0).reshape(D, 1).astype(bf)
    # rank-1 terms of the identity fold into alpha/beta
    alpha = pos @ (w1 + w3) + b  # [N]
    beta = np.ascontiguousarray(
        (allv @ (w2 - w3)).reshape(NJ, 1)
    ).astype(np.float32)

    in_maps = []
    for c in range(NCORES):
        sl = slice(c * IL, (c + 1) * IL)
        pos_loc = np.ascontiguousarray(pos[sl])
        pos_loc_s = pos_loc * aw3
        in_maps.append(
            {
                "pos": pos,
                "neg": neg,
                "pos_loc": pos_loc,
                "bst": bst,
                "gst": gst,
                "pst": np.ascontiguousarray(pos_loc.T).astype(bf),
                "pstf": np.ascontiguousarray(pos_loc_s.T).astype(np.float32),
                "pstfn": np.ascontiguousarray(-pos_loc_s.T).astype(np.float32),
                "w3t": w3t,
                "alpha_l": np.ascontiguousarray(
                    alpha[sl].reshape(IL, 1)
                ).astype(np.float32),
                "beta": beta,
            }
        )
    return in_maps


def kernel(tensor_positive, tensor_negative, linear_w, linear_b):
    import time

    from concourse.bass_utils import run_bass_kernel_spmd

    in_maps = _prep_inputs(tensor_positive, tensor_negative, linear_w, linear_b)
    if "nc" not in _CACHE:
        _CACHE["nc"] = _build_program()
    nc = _CACHE["nc"]
    # A NeuronCore occasionally comes up wedged from a previous run
    # (NRT_EXEC_UNIT_UNRECOVERABLE); it clears on retry.
    last_err = None
    for attempt in range(3):
        try:
            res = run_bass_kernel_spmd(nc, in_maps, core_ids=list(range(NCORES)))
            break
        except Exception as e:  # noqa: BLE001
            last_err = e
            if attempt == 2:
                raise
            time.sleep(20)
    total = np.float64(0.0)
    for c in range(NCORES):
        nce = np.asarray(res.results[c]["out_nce"], np.float64)
        bce = np.asarray(res.results[c]["out_bce"], np.float64)
        total += nce.sum() + bce.sum() / NJ
    return np.asarray(total, dtype=np.float32)
